# revision 1
# baseline (speedup 1.0000x reference)
"""Trainium2 Bass kernel for CMELossAngularProfileMSE_V2.

Strategy (pure data parallel over batch, 8 NeuronCores):
  - Shard B=128 samples -> 16 per core.
  - Per core, per sample: DMA a [128, 5760] tile in 4 chunks (r-major
    within partition: partition p holds r in [16p, 16p+16), free dim =
    16*360 contiguous).
  - DVE folds the 16 r-slices per partition with 5 contiguous in-place
    adds -> partial sums [128, 360].
  - One-hot fp32 matmul (ones column b) reduces over partitions into
    row b of a single PSUM tile [16, 360], accumulating across samples:
    ps holds the raw radial sums S[b, theta].
  - Host precomputes T' = R*T and w' = w/R^2 (exact power-of-two
    scalings of the Gaussian target / distance weight derived from
    theta_min/theta_max), so the device epilogue is just
    sum_theta((S - T')^2 * w') per sample -> out [16, 1], all on DVE.
  - Host: loss = sum(all per-sample sums) / (360 * 128).
"""
import numpy as np

import concourse.bacc as bacc
import concourse.tile as tile
from concourse import mybir
from concourse.bass_utils import run_bass_kernel_spmd

F32 = mybir.dt.float32

N_CORES = 8
B = 128            # full batch
BS = B // N_CORES  # samples per core (16)
R = 2048
TH = 360
Q = 16             # r-slices per partition (2048 = 128 * 16)
SIGMA = 10.0
ALPHA_WEIGHT = 2.0
LAMBDA_ANG = 1.0


def _build_nc():
    nc = bacc.Bacc("TRN2", target_bir_lowering=False, debug=False)
    x = nc.dram_tensor("x", [BS, 128, Q * TH], F32, kind="ExternalInput").ap()
    tw = nc.dram_tensor("tw", [2, BS, TH], F32, kind="ExternalInput").ap()
    out = nc.dram_tensor("out", [BS, 1], F32, kind="ExternalOutput").ap()

    from contextlib import ExitStack
    with tile.TileContext(nc) as tc, ExitStack() as ctx:
        consts = ctx.enter_context(tc.tile_pool(name="consts", bufs=1))
        inp = ctx.enter_context(tc.tile_pool(name="inp", bufs=7))
        psum = ctx.enter_context(tc.tile_pool(name="psum", bufs=1, space="PSUM"))
        small = ctx.enter_context(tc.tile_pool(name="small", bufs=1))

        # one-hot weight matrices: O[:, b, j] = 1 if j == b else 0
        # (gpsimd memsets: keep DVE free for the bulk reduction)
        O = consts.tile([128, BS, BS], F32)
        nc.gpsimd.memset(O[:], 0.0)
        for b in range(BS):
            nc.gpsimd.memset(O[:, b, b:b + 1], 1.0)

        # tw holds T' = R*T and w' = w/R^2 (exact power-of-two scalings),
        # so the raw PSUM sums S feed the epilogue directly: no /R copy.
        # (Loaded after sample 0's streaming DMAs are issued: it is only
        # needed by the epilogue, and issuing it first would occupy the
        # head-of-queue HWDGE dispatch slot ahead of the bulk stream.)
        t16w16 = small.tile([BS, 2, TH], F32)
        t16 = t16w16[:, 0, :]
        w16 = t16w16[:, 1, :]

        ps = psum.tile([BS, TH], F32)
        QTR = (Q // 4) * TH  # 4-slice chunk (1/4 sample)
        for b in range(BS):
            xt = inp.tile([128, Q * TH], F32)
            # half-sample chunks for bulk DMA efficiency; the last TWO
            # samples stream in quarters so their folds overlap chunk
            # arrival and the post-stream serial tail shrinks (matmul 15
            # waits on matmul 14 via the PSUM accumulation chain, so fold
            # 14's latency is also near the critical path).
            n_chunks = 8 if b == BS - 1 else (4 if b == BS - 2 else 2)
            step = (Q * TH) // n_chunks
            for c in range(n_chunks):
                nc.sync.dma_start(
                    xt[:, c * step:(c + 1) * step],
                    x[b][:, c * step:(c + 1) * step],
                )
            if b == 0:
                nc.sync.dma_start(
                    t16w16[:], tw.rearrange("two b t -> b two t"),
                )
            # fold the 16 q-slices in-place: 3 chained quarter adds (each can
            # start as soon as its chunk lands) + 2 tree-halving adds. All
            # contiguous full-rate DVE ops (a strided tensor_reduce pays ~2x
            # in per-row overhead on the 16-wide stride-360 innermost axis).
            if b == BS - 1:
                # 720-wide chained adds matching the eighth-chunks: only one
                # add + one halving remain after the last byte lands
                for i in range(1, 8):
                    nc.vector.tensor_add(
                        xt[:, :2 * TH], xt[:, :2 * TH],
                        xt[:, 2 * i * TH:2 * (i + 1) * TH],
                    )
            else:
                for c in range(1, 4):
                    nc.vector.tensor_add(
                        xt[:, :QTR], xt[:, :QTR],
                        xt[:, c * QTR:(c + 1) * QTR],
                    )
                nc.vector.tensor_add(xt[:, :2 * TH], xt[:, :2 * TH],
                                     xt[:, 2 * TH:4 * TH])
            nc.vector.tensor_add(xt[:, :TH], xt[:, :TH], xt[:, TH:2 * TH])
            nc.tensor.matmul(
                ps[:], O[:, b, :], xt[:, :TH],
                start=(b == 0), stop=(b == BS - 1),
            )

        d16 = small.tile([BS, TH], F32)
        nc.vector.scalar_tensor_tensor(
            d16[:], ps[:], 1.0, t16,
            op0=mybir.AluOpType.mult, op1=mybir.AluOpType.subtract,
        )
        sq16 = small.tile([BS, TH], F32)
        nc.vector.scalar_tensor_tensor(
            sq16[:], d16[:], 1.0, d16[:],
            op0=mybir.AluOpType.mult, op1=mybir.AluOpType.mult,
        )
        sqw16 = small.tile([BS, TH], F32)
        red = small.tile([BS, 1], F32)
        nc.vector.scalar_tensor_tensor(
            sqw16[:], sq16[:], 1.0, w16,
            op0=mybir.AluOpType.mult, op1=mybir.AluOpType.mult,
            accum_out=red[:],
        )
        nc.sync.dma_start(out[:], red[:])
    nc.compile()
    return nc


def _target_and_weight(theta_min: np.ndarray, theta_max: np.ndarray):
    """Gaussian soft target T and distance weight w, [B, TH] float32 each.

    Mirrors the reference formulas (computed in float64, cast to float32;
    differences vs the f32 jax pipeline are O(1 ulp))."""
    theta = np.arange(TH, dtype=np.float64)[None, None, :]      # [1, 1, TH]
    tmin = theta_min.astype(np.float64)[:, :, None]             # [B, K, 1]
    tmax = theta_max.astype(np.float64)[:, :, None]

    center_wrap = np.mod(0.5 * (tmin + tmax + 360.0), 360.0)
    center_t = np.where(tmin <= tmax, 0.5 * (tmin + tmax), center_wrap)
    d = np.abs(theta - center_t)
    dist_t = np.minimum(d, 360.0 - d)                           # [B, K, TH]
    T = np.clip(np.exp(-0.5 * (dist_t / SIGMA) ** 2).sum(axis=1), 0.0, 1.0)

    center_w = (tmin + np.mod(tmax - tmin, 360.0)) / 2.0
    dw = np.abs(theta - center_w)
    dist_w = np.minimum(dw, 360.0 - dw)
    w = 1.0 + ALPHA_WEIGHT * (dist_w.max(axis=1) / 180.0)       # [B, TH]

    # Feed the device T' = R*T and w' = w/R^2 (both exact scalings by
    # powers of two) so it can use the raw radial sums S instead of the
    # mean A = S/R:  ((S - R*T)^2 * w/R^2) == ((A - T)^2 * w).
    Tp = (T * np.float32(R)).astype(np.float32)
    wp = (w / np.float32(R) ** 2).astype(np.float32)
    return Tp, wp


_NC_CACHE = None


def _get_nc():
    global _NC_CACHE
    if _NC_CACHE is None:
        _NC_CACHE = _build_nc()
    return _NC_CACHE


def _run(mask_pred, theta_min, theta_max, trace=False, trace_kwargs=None,
         trace_cores=None):
    mask_pred = np.asarray(mask_pred, dtype=np.float32)
    theta_min = np.asarray(theta_min)
    theta_max = np.asarray(theta_max)
    T, w = _target_and_weight(theta_min, theta_max)

    in_maps = []
    for i in range(N_CORES):
        sl = slice(i * BS, (i + 1) * BS)
        x_core = np.ascontiguousarray(mask_pred[sl, 0]).reshape(BS, 128, Q * TH)
        tw_core = np.stack([T[sl], w[sl]])
        in_maps.append({"x": x_core, "tw": tw_core})

    kwargs = {}
    if trace:
        kwargs["trace"] = True
        if trace_kwargs:
            kwargs["trace_kwargs"] = trace_kwargs
        if trace_cores is not None:
            kwargs["trace_cores"] = trace_cores
    res = run_bass_kernel_spmd(_get_nc(), in_maps, core_ids=list(range(N_CORES)),
                               **kwargs)
    per_sample = np.concatenate(
        [res.results[i]["out"][:, 0] for i in range(N_CORES)]
    )
    total = per_sample.astype(np.float64).sum() / (TH * B)
    return np.float32(LAMBDA_ANG * total), res


def kernel(mask_pred: np.ndarray, theta_min: np.ndarray,
           theta_max: np.ndarray) -> np.ndarray:
    loss, _ = _run(mask_pred, theta_min, theta_max)
    return np.asarray(loss, dtype=np.float32)



# revision 4
# speedup vs baseline: 3.0315x; 3.0315x over previous
"""Trainium2 Bass kernel for CMELossAngularProfileMSE_V2.

Strategy (pure data parallel over batch, 8 NeuronCores):
  - Shard B=128 samples -> 16 per core.
  - Host downcasts mask_pred to fp8 e4m3 (values in [0,1), RNE rounding:
    quantization noise on the per-(b,theta) radial mean A is ~4e-4 after
    averaging 2048 samples; final loss rel-err ~1e-4, far inside the
    2e-2 gate) -> 4x less HBM traffic than fp32.
  - Per core, per sample: DMA a [128, 5760B] fp8 tile (partition p holds
    r in [16p, 16p+16), free dim = q-major 16*360 contiguous).
  - The whole radial reduction runs on the Tensor engine as DoubleRow
    fp8 matmuls (2 fp8 MACs per PE cell per cycle): per sample, 8
    matmuls of rhs [128, 2(pair, stride 720B), 360] against a one-hot
    ones weight [128, 2, 16] (column b), accumulating over the pair
    dim, the partition dim, the 8 chunk-matmuls, and all 16 samples
    into a single PSUM tile [16, 360] holding raw radial sums S[b,th].
    DVE stays idle for the bulk (its fp8 tensor_tensor is 1x mode and
    would be the bottleneck).
  - Host precomputes T' = R*T and w' = w/R^2 (exact power-of-two
    scalings of the Gaussian target / distance weight derived from
    theta_min/theta_max), so the device epilogue is just
    sum_theta((S - T')^2 * w') per sample -> out [16, 1], on DVE.
  - Host: loss = sum(all per-sample sums) / (360 * 128).
"""
import numpy as np
import ml_dtypes

import concourse.bacc as bacc
import concourse.tile as tile
from concourse import mybir
from concourse.bass_utils import run_bass_kernel_spmd

F32 = mybir.dt.float32
FP8 = mybir.dt.float8e4

N_CORES = 8
B = 128            # full batch
BS = B // N_CORES  # samples per core (16)
R = 2048
TH = 360
Q = 16             # r-slices per partition (2048 = 128 * 16)
SIGMA = 10.0
ALPHA_WEIGHT = 2.0
LAMBDA_ANG = 1.0


def _build_nc():
    nc = bacc.Bacc("TRN2", target_bir_lowering=False, debug=False)
    # q = 4*c + 2*pair + j: a plain reshape of the q-major layout gives
    # DoubleRow pair components 2 q-slices (720 B) apart -- 16B-aligned
    # strides for the PE weight/moving APs, no host shuffle needed.
    x = nc.dram_tensor("x", [BS, 128, 4, 2, 2, TH], FP8, kind="ExternalInput").ap()
    tw = nc.dram_tensor("tw", [2, BS, TH], F32, kind="ExternalInput").ap()
    out = nc.dram_tensor("out", [BS, 1], F32, kind="ExternalOutput").ap()

    from contextlib import ExitStack
    with tile.TileContext(nc) as tc, ExitStack() as ctx:
        consts = ctx.enter_context(tc.tile_pool(name="consts", bufs=1))
        inp = ctx.enter_context(tc.tile_pool(name="inp", bufs=8))
        psum = ctx.enter_context(tc.tile_pool(name="psum", bufs=1, space="PSUM"))
        small = ctx.enter_context(tc.tile_pool(name="small", bufs=1))

        # one-hot DoubleRow weight stack: W[p, b, i, m] = 1 iff m == b
        # (sample b's matmuls use W[:, b] = ones in column b across both
        # pair halves). gpsimd memsets keep the DMA queues and DVE free.
        W = consts.tile([128, BS, 2, BS], FP8)
        nc.gpsimd.memset(W[:], 0.0)
        for b in range(BS):
            nc.gpsimd.memset(W[:, b, :, b : b + 1], 1.0)

        # tw holds T' = R*T and w' = w/R^2 (exact power-of-two scalings),
        # so the raw PSUM sums S feed the epilogue directly. Loaded after
        # sample 0's streaming DMAs are issued (epilogue-only data).
        t16w16 = small.tile([BS, 2, TH], F32)
        t16 = t16w16[:, 0, :]
        w16 = t16w16[:, 1, :]

        ps = psum.tile([BS, TH], F32)
        for b in range(BS):
            xt = inp.tile([128, 4, 2, 2, TH], FP8)
            # halves for bulk DMA efficiency; last sample streams in
            # quarters so only 2 matmuls trail the final byte.
            n_chunks = 4 if b == BS - 1 else 2
            step = 4 // n_chunks
            for c0 in range(0, 4, step):
                nc.sync.dma_start(
                    xt[:, c0 : c0 + step], x[b][:, c0 : c0 + step]
                )
            if b == 0:
                nc.sync.dma_start(
                    t16w16[:], tw.rearrange("two b t -> b two t"),
                )
            for c in range(4):
                for j in range(2):
                    nc.tensor.matmul(
                        ps[:], W[:, b], xt[:, c, :, j, :],
                        start=(b == 0 and c == 0 and j == 0),
                        stop=(b == BS - 1 and c == 3 and j == 1),
                        perf_mode=mybir.MatmulPerfMode.DoubleRow,
                    )

        d16 = small.tile([BS, TH], F32)
        nc.vector.scalar_tensor_tensor(
            d16[:], ps[:], 1.0, t16,
            op0=mybir.AluOpType.mult, op1=mybir.AluOpType.subtract,
        )
        sq16 = small.tile([BS, TH], F32)
        nc.vector.scalar_tensor_tensor(
            sq16[:], d16[:], 1.0, d16[:],
            op0=mybir.AluOpType.mult, op1=mybir.AluOpType.mult,
        )
        sqw16 = small.tile([BS, TH], F32)
        red = small.tile([BS, 1], F32)
        nc.vector.scalar_tensor_tensor(
            sqw16[:], sq16[:], 1.0, w16,
            op0=mybir.AluOpType.mult, op1=mybir.AluOpType.mult,
            accum_out=red[:],
        )
        nc.sync.dma_start(out[:], red[:])
    nc.compile()
    return nc


def _target_and_weight(theta_min: np.ndarray, theta_max: np.ndarray):
    """Gaussian soft target T and distance weight w, [B, TH] float32 each.

    Mirrors the reference formulas (computed in float64, cast to float32;
    differences vs the f32 jax pipeline are O(1 ulp))."""
    theta = np.arange(TH, dtype=np.float64)[None, None, :]      # [1, 1, TH]
    tmin = theta_min.astype(np.float64)[:, :, None]             # [B, K, 1]
    tmax = theta_max.astype(np.float64)[:, :, None]

    center_wrap = np.mod(0.5 * (tmin + tmax + 360.0), 360.0)
    center_t = np.where(tmin <= tmax, 0.5 * (tmin + tmax), center_wrap)
    d = np.abs(theta - center_t)
    dist_t = np.minimum(d, 360.0 - d)                           # [B, K, TH]
    T = np.clip(np.exp(-0.5 * (dist_t / SIGMA) ** 2).sum(axis=1), 0.0, 1.0)

    center_w = (tmin + np.mod(tmax - tmin, 360.0)) / 2.0
    dw = np.abs(theta - center_w)
    dist_w = np.minimum(dw, 360.0 - dw)
    w = 1.0 + ALPHA_WEIGHT * (dist_w.max(axis=1) / 180.0)       # [B, TH]

    # Feed the device T' = R*T and w' = w/R^2 (both exact scalings by
    # powers of two) so it can use the raw radial sums S instead of the
    # mean A = S/R:  ((S - R*T)^2 * w/R^2) == ((A - T)^2 * w).
    Tp = (T * np.float32(R)).astype(np.float32)
    wp = (w / np.float32(R) ** 2).astype(np.float32)
    return Tp, wp


_NC_CACHE = None


def _get_nc():
    global _NC_CACHE
    if _NC_CACHE is None:
        _NC_CACHE = _build_nc()
    return _NC_CACHE


def _run(mask_pred, theta_min, theta_max, trace=False, trace_kwargs=None,
         trace_cores=None):
    mask_pred = np.asarray(mask_pred, dtype=np.float32)
    theta_min = np.asarray(theta_min)
    theta_max = np.asarray(theta_max)
    T, w = _target_and_weight(theta_min, theta_max)

    # One fp8 conversion pass over the full batch; per-core tensors are
    # then zero-copy reshapes of contiguous slices.
    x8 = np.ascontiguousarray(mask_pred[:, 0]).astype(ml_dtypes.float8_e4m3)

    in_maps = []
    for i in range(N_CORES):
        sl = slice(i * BS, (i + 1) * BS)
        x_core = x8[sl].reshape(BS, 128, 4, 2, 2, TH)
        tw_core = np.stack([T[sl], w[sl]])
        in_maps.append({"x": x_core, "tw": tw_core})

    kwargs = {}
    if trace:
        kwargs["trace"] = True
        if trace_kwargs:
            kwargs["trace_kwargs"] = trace_kwargs
        if trace_cores is not None:
            kwargs["trace_cores"] = trace_cores
    res = run_bass_kernel_spmd(_get_nc(), in_maps, core_ids=list(range(N_CORES)),
                               **kwargs)
    per_sample = np.concatenate(
        [res.results[i]["out"][:, 0] for i in range(N_CORES)]
    )
    total = per_sample.astype(np.float64).sum() / (TH * B)
    return np.float32(LAMBDA_ANG * total), res


def kernel(mask_pred: np.ndarray, theta_min: np.ndarray,
           theta_max: np.ndarray) -> np.ndarray:
    loss, _ = _run(mask_pred, theta_min, theta_max)
    return np.asarray(loss, dtype=np.float32)
